# revision 40
# baseline (speedup 1.0000x reference)
"""Causal self-attention (K/Q swapped variant) on 8 trn2 NeuronCores.

Sharding: core c = (b, g) with b = c // 4 (batch), g = c % 4 (head group of
4 heads).  Each core computes, for its batch and heads, the full attention
and a partial output projection (its heads' rows of Wproj); the host sums
the 4 partials per batch and adds bproj (+ the folded V-bias term
sum_h bv_h @ Wproj[h rows], which commutes through softmax since rows sum
to 1).

Per-core kernel (bf16 matmuls, fp32 PSUM accumulation):
  - x[b]^T arrives pre-transposed (and bf16-rounded) from host as [D, N].
  - K^T, Q^T per head-pair: [128, N] tiles (2 heads stacked on partitions),
    via W-stationary matmuls; k/q biases added during PSUM->SBUF eviction
    on DVE.
  - V is produced DIRECTLY in [n, dk] layout (x^T-chunk stationary, Wv
    moving) into per-head [m, 65] tiles whose 65th column is 1.0 (gives
    softmax row-sums for free in the O matmul).  No PE transposes.
  - S^T[m, n] = sum_d Q^T[d, m] K^T[d, n] = scores[n, m]; the two heads of
    a pair run row-packed (partitions 0-63 / 64-127) writing the two bank
    halves of one fused [128, 1024] PSUM tile, so one ACT exp covers both.
    Fully-masked tiles are skipped; diagonal-band tiles only compute the
    live column range.
  - E = exp(S / 8) on ACT (no max-subtraction: scores are O(1)); causal
    masking by multiplying diagonal-band tiles with 0/1 masks on DVE.
  - O_aug = V_aug^T . E accumulated over m-blocks: rows 0-63 are the
    unnormalized output^T, row 64 the softmax denominator.
  - normalize: reciprocal_approx_fast of row 64 (fp32, ~5x faster than the
    iterative divide), PE outer-product broadcast (float32r matmul, no cast
    op), then one DVE multiply reading both PSUM operands.
  - partial out (fp32) = sum_h O_h^T.T @ Wproj[head rows] in PSUM, emitted
    per n-block; kqv for block j+1 is interleaved between the two attention
    pair-phases of block j so the PE never idles long enough to re-throttle
    (HAM).
"""

import os
import sys

if "/opt/trn_rl_repo" not in sys.path:
    sys.path.insert(0, "/opt/trn_rl_repo")

import numpy as np

B, N, D, H = 2, 2048, 1024, 16
DK = 64
NCORES = 8
GROUPS = 4          # head groups
HPC = H // GROUPS   # 4 heads per core
CH = D // 128       # 8 contraction chunks
NB = N // 512       # 4 n-blocks
MBS = N // 128      # 16 m-blocks

_CACHE = {}


def _build_program():
    import concourse.bacc as bacc
    import concourse.mybir as mybir
    from concourse.tile import TileContext
    from contextlib import ExitStack

    # The act-table-load pass greedily binds each activation to the FIRST
    # table set containing it (exp -> exp_and_others, ln -> natural_log),
    # thrashing 2.7us table swaps at every softmax normalize.  Steer both
    # to the combined natural_log_exp_and_others set so one load serves all.
    orig_tables = bacc.get_activation_tables

    def _tables_one_set(arch):
        t = orig_tables(arch)
        exp_ln = {mybir.ActivationFunctionType.Exp,
                  mybir.ActivationFunctionType.Ln}
        return {
            name: (fns - exp_ln if name != "natural_log_exp_and_others" else fns)
            for name, fns in t.items()
        }

    bacc.get_activation_tables = _tables_one_set

    f32 = mybir.dt.float32
    f32r = mybir.dt.float32r
    bf = mybir.dt.bfloat16
    EXP = mybir.ActivationFunctionType.Exp
    LN = mybir.ActivationFunctionType.Ln

    nc = bacc.Bacc(
        "TRN2",
        target_bir_lowering=False,
        debug=False,
        enable_asserts=False,
        num_devices=NCORES,
    )

    xT = nc.dram_tensor("xT", [D, N], bf, kind="ExternalInput").ap()
    wk = nc.dram_tensor("wk", [CH, 128, 256], bf, kind="ExternalInput").ap()
    wq = nc.dram_tensor("wq", [CH, 128, 256], bf, kind="ExternalInput").ap()
    wv = nc.dram_tensor("wv", [CH, 128, 256], bf, kind="ExternalInput").ap()
    wp = nc.dram_tensor("wp", [2, 128, D], bf, kind="ExternalInput").ap()
    masks = nc.dram_tensor("masks", [128, 4 * 512], bf, kind="ExternalInput").ap()
    bias = nc.dram_tensor("bias", [128, 4], f32, kind="ExternalInput").ap()
    ones_d = nc.dram_tensor("ones", [128, 64], bf, kind="ExternalInput").ap()
    out_p = nc.dram_tensor("out_p", [N, D], bf, kind="ExternalOutput").ap()

    with TileContext(nc) as tc, ExitStack() as ctx:
        constp = ctx.enter_context(tc.tile_pool(name="const", bufs=1))
        storep = ctx.enter_context(tc.tile_pool(name="store", bufs=1))
        xtp = ctx.enter_context(tc.tile_pool(name="xt", bufs=4))
        ep = ctx.enter_context(tc.tile_pool(name="e", bufs=8))
        rcp = ctx.enter_context(tc.tile_pool(name="rc", bufs=4))
        oddp = ctx.enter_context(tc.tile_pool(name="odd", bufs=2))
        osp = ctx.enter_context(tc.tile_pool(name="os", bufs=3))
        # PSUM: o(2 banks) + s(2x2 banks) + kqv(2 banks) = 8
        pso = ctx.enter_context(tc.tile_pool(name="pso", bufs=2, space="PSUM"))
        pss = ctx.enter_context(tc.tile_pool(name="pss", bufs=2, space="PSUM"))
        psk = ctx.enter_context(tc.tile_pool(name="psk", bufs=2, space="PSUM"))

        # ---- constants / weights in SBUF ----
        wk_sb = constp.tile([128, CH * 256], bf, tag="wk")
        wq_sb = constp.tile([128, CH * 256], bf, tag="wq")
        wv_sb = constp.tile([128, CH * 256], bf, tag="wv")
        wp_sb = constp.tile([128, 2 * D], bf, tag="wp")
        masks_sb = constp.tile([128, 4 * 512], bf, tag="masks")
        bias_sb = constp.tile([128, 4], f32, tag="bias")
        ones_sb = constp.tile([128, 64], bf, tag="ones")

        # batched DMAs: one 3D-AP transfer per tensor (descriptor generation
        # on the sync queue costs ~600ns per dma_start, so fewer is faster)
        wk3 = wk_sb.rearrange("p (c f) -> p c f", c=CH)
        wq3 = wq_sb.rearrange("p (c f) -> p c f", c=CH)
        wv3 = wv_sb.rearrange("p (c f) -> p c f", c=CH)

        def load_x_block(jn, split=2):
            t = xtp.tile([128, CH * 512], bf, tag="xt", name=f"xtb{jn}")
            step = CH // split
            for i in range(split):
                nc.sync.dma_start(
                    t.rearrange("p (c n) -> p c n", c=CH)[
                        :, i * step:(i + 1) * step, :],
                    xT.rearrange("(c p) n -> p c n", p=128)[
                        :, i * step:(i + 1) * step, jn * 512:(jn + 1) * 512],
                )
            return [t[:, c * 512:(c + 1) * 512] for c in range(CH)]

        # prologue DMAs in consumption order (completion waits are monotonic
        # per queue, so anything issued before a consumer's data delays it):
        # K essentials first, then Q, V, masks; xt1/wp last
        xt0t = xtp.tile([128, CH * 512], bf, tag="xt", name="xtb0")
        x3 = xT.rearrange("(c p) n -> p c n", p=128)
        for c in range(CH):
            nc.sync.dma_start(
                xt0t.rearrange("p (c n) -> p c n", c=CH)[:, c:c + 1, :],
                x3[:, c:c + 1, 0:512],
            )
            nc.sync.dma_start(wk3[:, c:c + 1, :], wk.rearrange("c p f -> p c f")[:, c:c + 1, :])
        xt0 = [xt0t[:, c * 512:(c + 1) * 512] for c in range(CH)]
        nc.sync.dma_start(wq3, wq.rearrange("c p f -> p c f"))
        nc.sync.dma_start(wv3, wv.rearrange("c p f -> p c f"))
        nc.sync.dma_start(bias_sb[:], bias[:, :])
        nc.sync.dma_start(ones_sb[:], ones_d[:, :])
        nc.sync.dma_start(masks_sb[:], masks[:, :])
        xt1 = load_x_block(1)
        nc.sync.dma_start(
            wp_sb.rearrange("p (t d) -> p t d", t=2), wp.rearrange("t p d -> p t d")
        )

        # ---- persistent activation storage ----
        kt = storep.tile([128, 2 * N], bf, tag="kt")    # [pairfeat, pair*N + n]
        qt = storep.tile([128, 2 * N], bf, tag="qt")
        v_sb = [storep.tile([128, MBS * 65], bf, tag=f"v{h}", name=f"v{h}")
                for h in range(HPC)]
        otp = [storep.tile([128, N], bf, tag=f"otp{p}", name=f"otp{p}")
               for p in range(2)]
        ones_bf = ones_sb[64:65, 0:64]
        for h in range(HPC):
            nc.vector.tensor_copy(
                v_sb[h].rearrange("p (m c) -> p m c", c=65)[:, :, 64],
                ones_sb[:, 0:16],
            )

        xts = {0: xt0, 1: xt1}

        def kqv_pair(jn, pair):
            """Project k/q/v of n-block jn for head pair `pair`."""
            xt = xts[jn]
            for wsb, dst, bcol in ((wk_sb, kt, pair), (wq_sb, qt, 2 + pair)):
                ps = psk.tile([128, 512], f32, tag="kqv", name="pskq")
                for c in range(CH):
                    nc.tensor.matmul(
                        ps[:],
                        wsb[:, c * 256 + pair * 128: c * 256 + (pair + 1) * 128],
                        xt[c][:],
                        start=(c == 0),
                        stop=(c == CH - 1),
                    )
                nc.vector.tensor_scalar_add(
                    dst[:, pair * N + jn * 512: pair * N + (jn + 1) * 512],
                    ps[:],
                    bias_sb[:, bcol:bcol + 1],
                )
            # V directly in [n, dk] layout: x^T chunk stationary, Wv moving.
            # One bank-padded PSUM tile per 128-row sub-block so the DVE
            # eviction of one sub-block never reads a bank the PE is still
            # writing (PSUM bank collisions are fatal on HW).
            for sub in range(4):
                psv = psk.tile([128, 128], f32, tag="kqv", name="psv")
                for c in range(CH):
                    nc.tensor.matmul(
                        psv[:],
                        xt[c][:, sub * 128:(sub + 1) * 128],
                        wv_sb[:, c * 256 + pair * 128: c * 256 + (pair + 1) * 128],
                        start=(c == 0),
                        stop=(c == CH - 1),
                    )
                mb = jn * 4 + sub
                nc.vector.tensor_copy(
                    v_sb[2 * pair][:, mb * 65: mb * 65 + 64],
                    psv[:, 0:64],
                )
                nc.vector.tensor_copy(
                    v_sb[2 * pair + 1][:, mb * 65: mb * 65 + 64],
                    psv[:, 64:128],
                )

        def attention_pair(j, pair):
            """S^T, exp, mask, O accumulation for n-block j, head pair."""
            nm = 4 * j + 4
            o_ps = [
                pso.tile([65, 512], f32, tag="o", name=f"o{j}{pair}{hh}")
                for hh in range(2)
            ]
            for mb in range(nm):
                rdiag = mb - 4 * j
                c0 = 128 * rdiag if rdiag > 0 else 0
                w = 512 - c0
                s = pss.tile([128, 1024], f32, tag="s", name="s")
                for hh in range(2):
                    base = hh * 64
                    nc.tensor.matmul(
                        s[:, hh * 512 + c0: (hh + 1) * 512],
                        qt[base:base + 64,
                           pair * N + mb * 128: pair * N + (mb + 1) * 128],
                        kt[base:base + 64,
                           pair * N + j * 512 + c0: pair * N + (j + 1) * 512],
                    )
                e = ep.tile([128, 1024], bf, tag="e")
                s3 = s.rearrange("p (h n) -> p h n", h=2)[:, :, c0:512]
                e3 = e.rearrange("p (h n) -> p h n", h=2)[:, :, c0:512]
                nc.scalar.activation(e3, s3, EXP, scale=0.125)
                if rdiag >= 0:
                    for hh in range(2):
                        nc.vector.tensor_mul(
                            e[:, hh * 512 + c0:(hh + 1) * 512],
                            e[:, hh * 512 + c0:(hh + 1) * 512],
                            masks_sb[:, rdiag * 512 + c0:(rdiag + 1) * 512],
                        )
                for hh in range(2):
                    h = 2 * pair + hh
                    nc.tensor.matmul(
                        o_ps[hh][:, c0:512],
                        v_sb[h][:, mb * 65: mb * 65 + 65],
                        e[:, hh * 512 + c0:(hh + 1) * 512],
                        start=(mb == 0),
                        stop=(mb == nm - 1),
                    )
            return o_ps

        def denom_recip(j, pair, o_ps):
            """Evict unnormalized O to SBUF (frees the o PSUM slots fast) and
            compute 1/d = exp(-ln d) on ACT: Ln and Exp share one table set,
            so this is 2 fast ACT ops instead of a 3.3us serial DVE divide."""
            rcbs, onns = [], []
            for hh in range(2):
                onn = rcp.tile([64, 512], bf, tag="onn", name=f"onn{j}{pair}{hh}")
                nc.vector.tensor_copy(onn[:], o_ps[hh][0:64, :])
                ln_d = rcp.tile([65, 512], f32, tag="rc", name=f"rc{j}{pair}{hh}")
                nc.scalar.activation(ln_d[64:65, :], o_ps[hh][64:65, :], LN)
                rcb = rcp.tile([65, 512], bf, tag="rcb", name=f"rcb{j}{pair}{hh}")
                nc.scalar.activation(rcb[64:65, :], ln_d[64:65, :], EXP,
                                     scale=-1.0)
                rcbs.append(rcb)
                onns.append(onn)
            return rcbs, onns

        def norm_apply(j, pair, rcbs, onns):
            """Broadcast 1/d over 64 partitions, multiply -> otp rows.
            Emitted after the interleaved kqv matmuls so the PE broadcast
            never waits on the ACT reciprocal chain."""
            for hh in range(2):
                bc = pss.tile([64, 512], f32, tag="s", name="bc")
                nc.tensor.matmul(bc[:], ones_bf, rcbs[hh][64:65, :])
                bcs = rcp.tile([64, 512], bf, tag="bcs", name=f"bcs{j}{pair}{hh}")
                nc.vector.tensor_copy(bcs[:], bc[:])
                if hh == 0:
                    nc.vector.tensor_mul(
                        otp[pair][0:64, j * 512:(j + 1) * 512],
                        onns[hh][:],
                        bcs[:],
                    )
                else:
                    odd = oddp.tile([64, 512], bf, tag="odd")
                    nc.vector.tensor_mul(odd[:], onns[hh][:], bcs[:])
                    nc.sync.dma_start(
                        otp[pair][64:128, j * 512:(j + 1) * 512], odd[:]
                    )

        def proj_block(j):
            """Final projection for output rows of n-block j."""
            for sub in range(4):
                nbk = 4 * j + sub
                os_t = osp.tile([128, D], bf, tag="os")
                for cb in range(2):
                    fp = pss.tile([128, 512], f32, tag="s", name="fp")
                    for p2 in range(2):
                        nc.tensor.matmul(
                            fp[:],
                            otp[p2][:, nbk * 128:(nbk + 1) * 128],
                            wp_sb[:, p2 * D + cb * 512: p2 * D + (cb + 1) * 512],
                            start=(p2 == 0),
                            stop=(p2 == 1),
                        )
                    if sub % 2 == 0:
                        nc.scalar.copy(os_t[:, cb * 512:(cb + 1) * 512], fp[:])
                    else:
                        nc.vector.tensor_copy(os_t[:, cb * 512:(cb + 1) * 512], fp[:])
                nc.sync.dma_start(out_p[nbk * 128:(nbk + 1) * 128, :], os_t[:])

        # ---- prologue: kqv for block 0 (xt0 already in flight) ----
        for pair in range(2):
            kqv_pair(0, pair)

        for j in range(NB):
            pass

            o0 = attention_pair(j, 0)
            r0, n0 = denom_recip(j, 0, o0)
            if j + 1 < NB:
                kqv_pair(j + 1, 0)
            norm_apply(j, 0, r0, n0)
            o1 = attention_pair(j, 1)
            r1, n1 = denom_recip(j, 1, o1)
            if j + 1 < NB:
                kqv_pair(j + 1, 1)
            norm_apply(j, 1, r1, n1)
            proj_block(j)
            # prefetch x^T for block j+2 AFTER this block's odd/out DMAs so
            # they don't wait behind a 1MB transfer on the DMA counter
            if j + 2 < NB:
                xts[j + 2] = load_x_block(j + 2)

    try:
        nc.compile()
    finally:
        bacc.get_activation_tables = orig_tables
    return nc


def _get_program():
    if "nc" not in _CACHE:
        _CACHE["nc"] = _build_program()
    return _CACHE["nc"]


def _prep_in_maps(x, Wkqv, bkqv, Wproj, bproj):
    import ml_dtypes
    bf = ml_dtypes.bfloat16

    x = np.asarray(x, np.float32)
    Wkqv = np.asarray(Wkqv, np.float32)
    bkqv = np.asarray(bkqv, np.float32)
    Wproj = np.asarray(Wproj, np.float32)

    # de-interleave kqv columns: col 3d+0 -> k_d, 3d+1 -> q_d, 3d+2 -> v_d
    Wk = Wkqv[:, :, 0::3]  # [H, D, DK]
    Wq = Wkqv[:, :, 1::3]
    Wv = Wkqv[:, :, 2::3]
    bk = bkqv[:, 0::3]     # [H, DK]
    bq = bkqv[:, 1::3]

    masks = np.zeros((128, 4, 512), np.float32)
    mm = np.arange(128)[:, None]
    nn = np.arange(512)[None, :]
    for rr in range(4):
        masks[:, rr, :] = (128 * rr + mm <= nn).astype(np.float32)
    masks = np.ascontiguousarray(masks.reshape(128, 2048)).astype(bf)

    def wlayout(Wg):  # [4, D, DK] -> [CH, 128, 256] (pair-major columns)
        arr = Wg.reshape(2, 2, CH, 128, DK)          # [pair, hh, ch, p, f]
        return np.ascontiguousarray(
            arr.transpose(2, 3, 0, 1, 4).reshape(CH, 128, 256).astype(bf)
        )

    group_maps = []
    for g in range(GROUPS):
        hs = slice(g * HPC, (g + 1) * HPC)
        bias_t = np.zeros((128, 4), np.float32)
        for pair in range(2):
            h0, h1 = g * HPC + 2 * pair, g * HPC + 2 * pair + 1
            bias_t[0:64, pair] = bk[h0]
            bias_t[64:128, pair] = bk[h1]
            bias_t[0:64, 2 + pair] = bq[h0]
            bias_t[64:128, 2 + pair] = bq[h1]
        wp_c = np.ascontiguousarray(
            Wproj[g * HPC * DK:(g + 1) * HPC * DK].reshape(2, 128, D).astype(bf)
        )
        group_maps.append({
            "wk": wlayout(Wk[hs]),
            "wq": wlayout(Wq[hs]),
            "wv": wlayout(Wv[hs]),
            "wp": wp_c,
            "bias": bias_t,
            "masks": masks,
            "ones": np.ones((128, 64), bf),
        })

    xTs = [np.ascontiguousarray(x[b].T.astype(bf)) for b in range(B)]
    in_maps = []
    for c in range(NCORES):
        b, g = c // GROUPS, c % GROUPS
        m = dict(group_maps[g])
        m["xT"] = xTs[b]
        in_maps.append(m)
    return in_maps


def _run(inputs, trace=False):
    from concourse.bass_utils import run_bass_kernel_spmd

    nc = _get_program()
    in_maps = _prep_in_maps(
        inputs["x"], inputs["Wkqv"], inputs["bkqv"], inputs["Wproj"], inputs["bproj"]
    )
    res = run_bass_kernel_spmd(nc, in_maps, core_ids=list(range(NCORES)), trace=trace)
    bproj = np.asarray(inputs["bproj"], np.float32)
    # fold the V-bias through softmax (rows sum to 1): + sum_h bv_h @ Wproj[h]
    bkqv = np.asarray(inputs["bkqv"], np.float32)
    Wproj = np.asarray(inputs["Wproj"], np.float32)
    bv_flat = bkqv[:, 2::3].reshape(-1)  # [H*DK] = [D]
    bias_full = bproj + bv_flat @ Wproj
    out = np.empty((B, N, D), np.float32)
    for b in range(B):
        acc = res.results[b * GROUPS]["out_p"].astype(np.float32)
        for g in range(1, GROUPS):
            acc = acc + res.results[b * GROUPS + g]["out_p"]
        out[b] = acc + bias_full[None, :]
    return out, res


def kernel(**inputs):
    return _run(inputs)[0]


# revision 50
# speedup vs baseline: 1.2138x; 1.2138x over previous
"""Causal self-attention (K/Q swapped variant) on 8 trn2 NeuronCores.

Sharding: core c = (b, g) with b = c // 4 (batch), g = c % 4 (head group of
4 heads).  Each core computes, for its batch and heads, the full attention
and a partial output projection (its heads' rows of Wproj); the host sums
the 4 partials per batch and adds bproj (+ the folded V-bias term
sum_h bv_h @ Wproj[h rows], which commutes through softmax since rows sum
to 1).

Per-core kernel (bf16 matmuls, fp32 PSUM accumulation):
  - x[b]^T arrives pre-transposed (and bf16-rounded) from host as [D, N].
  - K^T, Q^T per head-pair: [128, N] tiles (2 heads stacked on partitions),
    via W-stationary matmuls; k/q biases added during PSUM->SBUF eviction
    on DVE.
  - V is produced DIRECTLY in [n, dk] layout (x^T-chunk stationary, Wv
    moving) into per-head [m, 65] tiles whose 65th column is 1.0 (gives
    softmax row-sums for free in the O matmul).  No PE transposes.
  - S^T[m, n] = sum_d Q^T[d, m] K^T[d, n] = scores[n, m]; the two heads of
    a pair run row-packed (partitions 0-63 / 64-127) writing the two bank
    halves of one fused [128, 1024] PSUM tile, so one ACT exp covers both.
    Fully-masked tiles are skipped; diagonal-band tiles only compute the
    live column range.
  - E = exp(S / 8) on ACT (no max-subtraction: scores are O(1)); causal
    masking by multiplying diagonal-band tiles with 0/1 masks on DVE.
  - O_aug = V_aug^T . E accumulated over m-blocks: rows 0-63 are the
    unnormalized output^T, row 64 the softmax denominator.
  - normalize: reciprocal_approx_fast of row 64 (fp32, ~5x faster than the
    iterative divide), PE outer-product broadcast (float32r matmul, no cast
    op), then one DVE multiply reading both PSUM operands.
  - partial out (fp32) = sum_h O_h^T.T @ Wproj[head rows] in PSUM, emitted
    per n-block; kqv for block j+1 is interleaved between the two attention
    pair-phases of block j so the PE never idles long enough to re-throttle
    (HAM).
"""

import os
import sys

if "/opt/trn_rl_repo" not in sys.path:
    sys.path.insert(0, "/opt/trn_rl_repo")

import numpy as np

B, N, D, H = 2, 2048, 1024, 16
DK = 64
NCORES = 8
GROUPS = 4          # head groups
HPC = H // GROUPS   # 4 heads per core
CH = D // 128       # 8 contraction chunks
NB = N // 512       # 4 n-blocks
MBS = N // 128      # 16 m-blocks

_CACHE = {}


def _build_program():
    import concourse.bacc as bacc
    import concourse.mybir as mybir
    from concourse.tile import TileContext
    from contextlib import ExitStack

    # The act-table-load pass greedily binds each activation to the FIRST
    # table set containing it (exp -> exp_and_others, ln -> natural_log),
    # thrashing 2.7us table swaps at every softmax normalize.  Steer both
    # to the combined natural_log_exp_and_others set so one load serves all.
    orig_tables = bacc.get_activation_tables

    def _tables_one_set(arch):
        t = orig_tables(arch)
        exp_ln = {mybir.ActivationFunctionType.Exp,
                  mybir.ActivationFunctionType.Ln}
        return {
            name: (fns - exp_ln if name != "natural_log_exp_and_others" else fns)
            for name, fns in t.items()
        }

    bacc.get_activation_tables = _tables_one_set

    f32 = mybir.dt.float32
    f32r = mybir.dt.float32r
    bf = mybir.dt.bfloat16
    EXP = mybir.ActivationFunctionType.Exp
    LN = mybir.ActivationFunctionType.Ln

    nc = bacc.Bacc(
        "TRN2",
        target_bir_lowering=False,
        debug=False,
        enable_asserts=False,
        num_devices=NCORES,
    )

    xT = nc.dram_tensor("xT", [D, N], bf, kind="ExternalInput").ap()
    wk = nc.dram_tensor("wk", [CH, 128, 256], bf, kind="ExternalInput").ap()
    wq = nc.dram_tensor("wq", [CH, 128, 256], bf, kind="ExternalInput").ap()
    wv = nc.dram_tensor("wv", [CH, 128, 256], bf, kind="ExternalInput").ap()
    wp = nc.dram_tensor("wp", [2, 128, D], bf, kind="ExternalInput").ap()
    masks = nc.dram_tensor("masks", [128, 4 * 512], bf, kind="ExternalInput").ap()
    bias = nc.dram_tensor("bias", [128, 4], f32, kind="ExternalInput").ap()
    ones_d = nc.dram_tensor("ones", [128, 64], bf, kind="ExternalInput").ap()
    out_p = nc.dram_tensor("out_p", [N, D], bf, kind="ExternalOutput").ap()

    with TileContext(nc) as tc, ExitStack() as ctx:
        constp = ctx.enter_context(tc.tile_pool(name="const", bufs=1))
        storep = ctx.enter_context(tc.tile_pool(name="store", bufs=1))
        xtp = ctx.enter_context(tc.tile_pool(name="xt", bufs=4))
        ep = ctx.enter_context(tc.tile_pool(name="e", bufs=8))
        rcp = ctx.enter_context(tc.tile_pool(name="rc", bufs=4))
        oddp = ctx.enter_context(tc.tile_pool(name="odd", bufs=2))
        osp = ctx.enter_context(tc.tile_pool(name="os", bufs=3))
        # PSUM: o(2 banks) + s(2x2 banks) + kqv(2 banks) = 8
        pso = ctx.enter_context(tc.tile_pool(name="pso", bufs=2, space="PSUM"))
        pss = ctx.enter_context(tc.tile_pool(name="pss", bufs=2, space="PSUM"))
        psk = ctx.enter_context(tc.tile_pool(name="psk", bufs=2, space="PSUM"))

        # ---- constants / weights in SBUF ----
        wk_sb = constp.tile([128, CH * 256], bf, tag="wk")
        wq_sb = constp.tile([128, CH * 256], bf, tag="wq")
        wv_sb = constp.tile([128, CH * 256], bf, tag="wv")
        wp_sb = constp.tile([128, 2 * D], bf, tag="wp")
        masks_sb = constp.tile([128, 4 * 512], bf, tag="masks")
        bias_sb = constp.tile([128, 4], f32, tag="bias")
        ones_sb = constp.tile([128, 64], bf, tag="ones")

        # batched DMAs: one 3D-AP transfer per tensor (descriptor generation
        # on the sync queue costs ~600ns per dma_start, so fewer is faster)
        wk3 = wk_sb.rearrange("p (c f) -> p c f", c=CH)
        wq3 = wq_sb.rearrange("p (c f) -> p c f", c=CH)
        wv3 = wv_sb.rearrange("p (c f) -> p c f", c=CH)

        def load_x_block(jn, split=2):
            t = xtp.tile([128, CH * 512], bf, tag="xt", name=f"xtb{jn}")
            step = CH // split
            for i in range(split):
                nc.sync.dma_start(
                    t.rearrange("p (c n) -> p c n", c=CH)[
                        :, i * step:(i + 1) * step, :],
                    xT.rearrange("(c p) n -> p c n", p=128)[
                        :, i * step:(i + 1) * step, jn * 512:(jn + 1) * 512],
                )
            return [t[:, c * 512:(c + 1) * 512] for c in range(CH)]

        xt0 = load_x_block(0, split=1)
        nc.sync.dma_start(wk3, wk.rearrange("c p f -> p c f"))
        nc.sync.dma_start(wq3, wq.rearrange("c p f -> p c f"))
        nc.sync.dma_start(wv3, wv.rearrange("c p f -> p c f"))
        nc.sync.dma_start(bias_sb[:], bias[:, :])
        nc.sync.dma_start(ones_sb[:], ones_d[:, :])
        nc.sync.dma_start(masks_sb[:], masks[:, :])
        xt1 = load_x_block(1, split=1)
        nc.sync.dma_start(
            wp_sb.rearrange("p (t d) -> p t d", t=2), wp.rearrange("t p d -> p t d")
        )

        # ---- persistent activation storage ----
        kt = storep.tile([128, 2 * N], bf, tag="kt")    # [pairfeat, pair*N + n]
        qt = storep.tile([128, 2 * N], bf, tag="qt")
        v_sb = [storep.tile([128, MBS * 65], bf, tag=f"v{h}", name=f"v{h}")
                for h in range(HPC)]
        otp = [storep.tile([128, N], bf, tag=f"otp{p}", name=f"otp{p}")
               for p in range(2)]
        ones_bf = ones_sb[64:65, 0:64]
        for h in range(HPC):
            nc.vector.tensor_copy(
                v_sb[h].rearrange("p (m c) -> p m c", c=65)[:, :, 64],
                ones_sb[:, 0:16],
            )

        xts = {0: xt0, 1: xt1}

        def kqv_pair(jn, pair):
            """Project k/q/v of n-block jn for head pair `pair`."""
            xt = xts[jn]
            for wsb, dst, bcol in ((wk_sb, kt, pair), (wq_sb, qt, 2 + pair)):
                ps = psk.tile([128, 512], f32, tag="kqv", name="pskq")
                for c in range(CH):
                    nc.tensor.matmul(
                        ps[:],
                        wsb[:, c * 256 + pair * 128: c * 256 + (pair + 1) * 128],
                        xt[c][:],
                        start=(c == 0),
                        stop=(c == CH - 1),
                    )
                nc.vector.tensor_scalar_add(
                    dst[:, pair * N + jn * 512: pair * N + (jn + 1) * 512],
                    ps[:],
                    bias_sb[:, bcol:bcol + 1],
                )
            # V directly in [n, dk] layout: x^T chunk stationary, Wv moving.
            # One bank-padded PSUM tile per 128-row sub-block so the DVE
            # eviction of one sub-block never reads a bank the PE is still
            # writing (PSUM bank collisions are fatal on HW).
            for sub in range(4):
                psv = psk.tile([128, 128], f32, tag="kqv", name="psv")
                for c in range(CH):
                    nc.tensor.matmul(
                        psv[:],
                        xt[c][:, sub * 128:(sub + 1) * 128],
                        wv_sb[:, c * 256 + pair * 128: c * 256 + (pair + 1) * 128],
                        start=(c == 0),
                        stop=(c == CH - 1),
                    )
                mb = jn * 4 + sub
                nc.vector.tensor_copy(
                    v_sb[2 * pair][:, mb * 65: mb * 65 + 64],
                    psv[:, 0:64],
                )
                nc.vector.tensor_copy(
                    v_sb[2 * pair + 1][:, mb * 65: mb * 65 + 64],
                    psv[:, 64:128],
                )

        def attention_pair(j, pair):
            """S^T, exp, mask, O accumulation for n-block j, head pair."""
            nm = 4 * j + 4
            o_ps = [
                pso.tile([65, 512], f32, tag="o", name=f"o{j}{pair}{hh}")
                for hh in range(2)
            ]
            for mb in range(nm):
                rdiag = mb - 4 * j
                c0 = 128 * rdiag if rdiag > 0 else 0
                w = 512 - c0
                s = pss.tile([128, 1024], f32, tag="s", name="s")
                for hh in range(2):
                    base = hh * 64
                    nc.tensor.matmul(
                        s[:, hh * 512 + c0: (hh + 1) * 512],
                        qt[base:base + 64,
                           pair * N + mb * 128: pair * N + (mb + 1) * 128],
                        kt[base:base + 64,
                           pair * N + j * 512 + c0: pair * N + (j + 1) * 512],
                    )
                e = ep.tile([128, 1024], bf, tag="e")
                s3 = s.rearrange("p (h n) -> p h n", h=2)[:, :, c0:512]
                e3 = e.rearrange("p (h n) -> p h n", h=2)[:, :, c0:512]
                nc.scalar.activation(e3, s3, EXP, scale=0.125)
                if rdiag >= 0:
                    for hh in range(2):
                        nc.vector.tensor_mul(
                            e[:, hh * 512 + c0:(hh + 1) * 512],
                            e[:, hh * 512 + c0:(hh + 1) * 512],
                            masks_sb[:, rdiag * 512 + c0:(rdiag + 1) * 512],
                        )
                for hh in range(2):
                    h = 2 * pair + hh
                    nc.tensor.matmul(
                        o_ps[hh][:, c0:512],
                        v_sb[h][:, mb * 65: mb * 65 + 65],
                        e[:, hh * 512 + c0:(hh + 1) * 512],
                        start=(mb == 0),
                        stop=(mb == nm - 1),
                    )
            return o_ps

        def denom_recip(j, pair, o_ps):
            """Evict unnormalized O to SBUF (frees the o PSUM slots fast) and
            compute 1/d = exp(-ln d) on ACT: Ln and Exp share one table set,
            so this is 2 fast ACT ops instead of a 3.3us serial DVE divide."""
            rcbs, onns = [], []
            for hh in range(2):
                onn = rcp.tile([64, 512], bf, tag="onn", name=f"onn{j}{pair}{hh}")
                nc.vector.tensor_copy(onn[:], o_ps[hh][0:64, :])
                ln_d = rcp.tile([65, 512], f32, tag="rc", name=f"rc{j}{pair}{hh}")
                nc.scalar.activation(ln_d[64:65, :], o_ps[hh][64:65, :], LN)
                rcb = rcp.tile([65, 512], bf, tag="rcb", name=f"rcb{j}{pair}{hh}")
                nc.scalar.activation(rcb[64:65, :], ln_d[64:65, :], EXP,
                                     scale=-1.0)
                rcbs.append(rcb)
                onns.append(onn)
            return rcbs, onns

        def norm_apply(j, pair, rcbs, onns):
            """Broadcast 1/d over 64 partitions, multiply -> otp rows.
            Emitted after the interleaved kqv matmuls so the PE broadcast
            never waits on the ACT reciprocal chain."""
            for hh in range(2):
                bc = pss.tile([64, 512], f32, tag="s", name="bc")
                nc.tensor.matmul(bc[:], ones_bf, rcbs[hh][64:65, :])
                bcs = rcp.tile([64, 512], bf, tag="bcs", name=f"bcs{j}{pair}{hh}")
                nc.vector.tensor_copy(bcs[:], bc[:])
                if hh == 0:
                    nc.vector.tensor_mul(
                        otp[pair][0:64, j * 512:(j + 1) * 512],
                        onns[hh][:],
                        bcs[:],
                    )
                else:
                    odd = oddp.tile([64, 512], bf, tag="odd")
                    nc.vector.tensor_mul(odd[:], onns[hh][:], bcs[:])
                    nc.sync.dma_start(
                        otp[pair][64:128, j * 512:(j + 1) * 512], odd[:]
                    )

        def proj_block(j):
            """Final projection for output rows of n-block j."""
            for sub in range(4):
                nbk = 4 * j + sub
                os_t = osp.tile([128, D], bf, tag="os")
                for cb in range(2):
                    fp = pss.tile([128, 512], f32, tag="s", name="fp")
                    for p2 in range(2):
                        nc.tensor.matmul(
                            fp[:],
                            otp[p2][:, nbk * 128:(nbk + 1) * 128],
                            wp_sb[:, p2 * D + cb * 512: p2 * D + (cb + 1) * 512],
                            start=(p2 == 0),
                            stop=(p2 == 1),
                        )
                    nc.vector.tensor_copy(os_t[:, cb * 512:(cb + 1) * 512], fp[:])
                nc.sync.dma_start(out_p[nbk * 128:(nbk + 1) * 128, :], os_t[:])

        # ---- prologue: kqv for block 0 (xt0 already in flight) ----
        for pair in range(2):
            kqv_pair(0, pair)

        for j in range(NB):
            if j + 2 < NB:
                xts[j + 2] = load_x_block(j + 2, split=1)

            o0 = attention_pair(j, 0)
            r0, n0 = denom_recip(j, 0, o0)
            if j + 1 < NB:
                kqv_pair(j + 1, 0)
            norm_apply(j, 0, r0, n0)
            o1 = attention_pair(j, 1)
            r1, n1 = denom_recip(j, 1, o1)
            if j + 1 < NB:
                kqv_pair(j + 1, 1)
            norm_apply(j, 1, r1, n1)
            proj_block(j)

    try:
        nc.compile()
    finally:
        bacc.get_activation_tables = orig_tables
    return nc


def _get_program():
    if "nc" not in _CACHE:
        _CACHE["nc"] = _build_program()
    return _CACHE["nc"]


def _prep_in_maps(x, Wkqv, bkqv, Wproj, bproj):
    import ml_dtypes
    bf = ml_dtypes.bfloat16

    x = np.asarray(x, np.float32)
    Wkqv = np.asarray(Wkqv, np.float32)
    bkqv = np.asarray(bkqv, np.float32)
    Wproj = np.asarray(Wproj, np.float32)

    # de-interleave kqv columns: col 3d+0 -> k_d, 3d+1 -> q_d, 3d+2 -> v_d
    Wk = Wkqv[:, :, 0::3]  # [H, D, DK]
    Wq = Wkqv[:, :, 1::3]
    Wv = Wkqv[:, :, 2::3]
    bk = bkqv[:, 0::3]     # [H, DK]
    bq = bkqv[:, 1::3]

    masks = np.zeros((128, 4, 512), np.float32)
    mm = np.arange(128)[:, None]
    nn = np.arange(512)[None, :]
    for rr in range(4):
        masks[:, rr, :] = (128 * rr + mm <= nn).astype(np.float32)
    masks = np.ascontiguousarray(masks.reshape(128, 2048)).astype(bf)

    def wlayout(Wg):  # [4, D, DK] -> [CH, 128, 256] (pair-major columns)
        arr = Wg.reshape(2, 2, CH, 128, DK)          # [pair, hh, ch, p, f]
        return np.ascontiguousarray(
            arr.transpose(2, 3, 0, 1, 4).reshape(CH, 128, 256).astype(bf)
        )

    group_maps = []
    for g in range(GROUPS):
        hs = slice(g * HPC, (g + 1) * HPC)
        bias_t = np.zeros((128, 4), np.float32)
        for pair in range(2):
            h0, h1 = g * HPC + 2 * pair, g * HPC + 2 * pair + 1
            bias_t[0:64, pair] = bk[h0]
            bias_t[64:128, pair] = bk[h1]
            bias_t[0:64, 2 + pair] = bq[h0]
            bias_t[64:128, 2 + pair] = bq[h1]
        wp_c = np.ascontiguousarray(
            Wproj[g * HPC * DK:(g + 1) * HPC * DK].reshape(2, 128, D).astype(bf)
        )
        group_maps.append({
            "wk": wlayout(Wk[hs]),
            "wq": wlayout(Wq[hs]),
            "wv": wlayout(Wv[hs]),
            "wp": wp_c,
            "bias": bias_t,
            "masks": masks,
            "ones": np.ones((128, 64), bf),
        })

    xTs = [np.ascontiguousarray(x[b].T.astype(bf)) for b in range(B)]
    in_maps = []
    for c in range(NCORES):
        b, g = c // GROUPS, c % GROUPS
        m = dict(group_maps[g])
        m["xT"] = xTs[b]
        in_maps.append(m)
    return in_maps


def _run(inputs, trace=False):
    from concourse.bass_utils import run_bass_kernel_spmd

    nc = _get_program()
    in_maps = _prep_in_maps(
        inputs["x"], inputs["Wkqv"], inputs["bkqv"], inputs["Wproj"], inputs["bproj"]
    )
    res = run_bass_kernel_spmd(nc, in_maps, core_ids=list(range(NCORES)), trace=trace)
    bproj = np.asarray(inputs["bproj"], np.float32)
    # fold the V-bias through softmax (rows sum to 1): + sum_h bv_h @ Wproj[h]
    bkqv = np.asarray(inputs["bkqv"], np.float32)
    Wproj = np.asarray(inputs["Wproj"], np.float32)
    bv_flat = bkqv[:, 2::3].reshape(-1)  # [H*DK] = [D]
    bias_full = bproj + bv_flat @ Wproj
    out = np.empty((B, N, D), np.float32)
    for b in range(B):
        acc = res.results[b * GROUPS]["out_p"].astype(np.float32)
        for g in range(1, GROUPS):
            acc = acc + res.results[b * GROUPS + g]["out_p"]
        out[b] = acc + bias_full[None, :]
    return out, res


def kernel(**inputs):
    return _run(inputs)[0]
